# revision 24
# baseline (speedup 1.0000x reference)
"""Trainium2 Bass kernel for nn_BiSVM: out[b,o] = diag(L @ x[b] @ R).

Math: out[b,o] = sum_{i,j} L[o,i] * x[b,i,j] * R[j,o]
  step 1 (TensorE): lx[o,j] = sum_i LT[i,o]^T @ x[b,i,j]   (LT = L^T, stationary)
  step 2 (VectorE): out[b,o] = sum_j lx[o,j] * RT[o,j]      (RT = R^T, fused
          multiply+reduce via scalar_tensor_tensor accum_out)

Sharding: data-parallel over batch, 8 batches per core on 8 NeuronCores;
L/R replicated. x and L are cast to fp16 on the host (PE matmul runs fp16 at
full rate, 1 cycle/row; products are exact, accumulation is fp32 in PSUM —
end-to-end error ~3e-4 relative to the fp32 reference). R stays fp32 in the
vector-engine reduction.

Self-contained: hardcodes shapes B=64, I=O=J=1024, 8 cores.

Optimization notes (all hardware-measured, loop-slope timing; run-to-run
noise is ~±2%):
  - This structure runs ~294-299 us = ~287 ns per N=512 fp16 matmul vs the
    216 ns pure-stream floor. Ablation (no-DVE / no-DMA / neither):
    284.7-293.5 us — concurrent DVE drains + x-DMA explain only ~10 us; the
    bare matmul pattern costs ~278 ns/MM, matching an isolated microbench of
    the same pattern (LDW per 2 MMs). A 248 ns/MM microbench shape (8
    same-weight matmuls into one psum bank per weight load) is unreachable
    here: accumulation chains must rotate weights every matmul (K=128/pass).
  - Neutral (no change): LDWEIGHTS dedupe (755 removed), semaphore-inc
    thinning 1024->64, forced it-major order (4 MMs/LDW), single-bank psum
    chains, it-major cold-start DMA ordering. Worse: rt in bf16 (303 us).
    Pool-engine drains are illegal (Pool=GPSIMD, no PSUM access).
  - Kept here: next-block x-DMA paced across the ot loop (2 tiles per step
    instead of a 4 MB burst); bf16 for the drain's scratch output (sc0 is
    write-only scratch; only accum_out is consumed); and just-in-time rt
    chunk loads in the block-0 cold start (rt[0] early, rt[1..7] after x) —
    the rt ordering is the one measurable win: the first drain needs rt[0]
    at ~10 us but the monolithic 4 MB rt DMA previously landed at ~28 us,
    backing up PSUM and stalling the PE (sim -16.6 us; HW 294-299 us ->
    280-289 us across three runs). Splitting the sp=0 loads finer was
    sim-negative (+1.9 us) and was reverted. Full per-MM accounting at this
    point: 216 stream + ~26 SEQ dispatch + ~26 LDW-per-2-MMs = 268.5 ns/MM
    + ~11.6 us edges = ~286 us — every term measured, nothing left that
    does not require fp8 (error-blocked) or violating the bipartite
    weight/bank constraint.
  - fp8 e4m3 DoubleRow (2x PE rate, K=256/pass) is error-blocked: measured
    3.9e-2 (both operands) / 2.65e-2 (x only) vs the 2e-2 gate; the minimum
    useful hybrid (1/4 of i) measures 1.94e-2 — only a 3% margin. No int8
    matmul exists in Bass (it would pass at ~1.3e-2).
"""

import numpy as np

import concourse.bacc as bacc
import concourse.mybir as mybir
import concourse.tile as tile
from concourse.bass_utils import run_bass_kernel_spmd

B, I, O, J = 64, 1024, 1024, 1024
NCORES = 8
BPC = B // NCORES          # batches per core
BBLK = 2                   # batches per SBUF-resident block
NBLK = BPC // BBLK
NOT = O // 128             # o-tiles
NIT = I // 128             # i-tiles (contraction)
NJC = J // 512             # j-chunks (psum bank width)

f16 = mybir.dt.float16
f32 = mybir.dt.float32


def _thin_sem_incs(nc, min_updates=256):
    """Drop per-matmul semaphore increments that no wait ever lands on.
    Keeps exactly the inc whose cumulative value first satisfies each waited
    value (every wait fires at the same instruction-completion as before),
    then renumbers all waits on that semaphore into the compressed counting
    (each kept inc stays +1). Verified numerics-identical on hardware."""
    import bisect
    from collections import defaultdict
    fn = nc.m.functions[0]
    blocks = list(fn.blocks)
    waited, bad = {}, set()
    for b in blocks:
        for i in b.instructions:
            si = i.sync_info
            if si is None:
                continue
            for w in si.on_wait:
                if w.ant_name is None:
                    continue
                if w.wait_mode != "sem-ge-imm" or w.wait_reg is not None:
                    bad.add(w.ant_name)
                else:
                    waited.setdefault(w.ant_name, set()).add(w.wait_value)
            for u in si.on_update:
                if u.ant_name is None:
                    continue
                if u.update_mode != "sem-inc" or u.update_reg is not None:
                    bad.add(u.ant_name)
    inc_block, incs, cum = {}, defaultdict(list), defaultdict(int)
    for b in blocks:
        for i in b.instructions:
            si = i.sync_info
            if si is None:
                continue
            for u in si.on_update:
                sm = u.ant_name
                if sm is None or sm in bad:
                    continue
                if sm in inc_block and inc_block[sm] != b.name:
                    bad.add(sm)
                    continue
                inc_block[sm] = b.name
                cum[sm] += u.update_value
                incs[sm].append((i, u, cum[sm]))
    ndrop = 0
    for sem, lst in incs.items():
        if sem in bad or len(lst) < min_updates:
            continue
        wvals = sorted(waited.get(sem, set()))
        keep = [False] * len(lst)
        keep[-1] = True
        it = iter(wvals)
        nxt = next(it, None)
        for k, (_, _, c) in enumerate(lst):
            while nxt is not None and c >= nxt:
                keep[k] = True
                nxt = next(it, None)
        kept_cums = [c for k, (_, _, c) in enumerate(lst) if keep[k]]
        for k, (inst, u, _) in enumerate(lst):
            if keep[k]:
                continue
            si = inst.sync_info
            ups = [x for x in si.on_update if x is not u]
            inst.sync_info = mybir.SyncInfo(
                on_wait=list(si.on_wait), on_update=ups)
            ndrop += 1
        for b in blocks:
            for i in b.instructions:
                si = i.sync_info
                if si is None or not any(
                        w.ant_name == sem for w in si.on_wait):
                    continue
                new_ws = []
                for w in si.on_wait:
                    if w.ant_name == sem:
                        new_ws.append(mybir.SyncWait(
                            sync_type=w.sync_type, id=w.id,
                            ant_name=w.ant_name, wait_mode=w.wait_mode,
                            wait_value=bisect.bisect_left(
                                kept_cums, w.wait_value) + 1))
                    else:
                        new_ws.append(w)
                i.sync_info = mybir.SyncInfo(
                    on_wait=new_ws, on_update=list(i.sync_info.on_update))
    return ndrop


def build_nc(reps: int | None = None):
    nc = bacc.Bacc("TRN2", target_bir_lowering=False, debug=False)
    x_d = nc.dram_tensor("x", [BPC, I, J], f16, kind="ExternalInput")
    lt_d = nc.dram_tensor("lt", [I, O], f16, kind="ExternalInput")
    rt_d = nc.dram_tensor("rt", [O, J], f32, kind="ExternalInput")
    # out_sb layout: [o_within_tile(128), ot(8) * b(8)] ; host reassembles
    out_d = nc.dram_tensor("out", [128, NOT * BPC], f32, kind="ExternalOutput")

    import contextlib

    def body(tc, wpool, xpool, spool, pspool):
            lt_sb = wpool.tile([128, NIT, O], f16, name="lt_sb")

            def load_lt_chunk(lts):
                nc.sync.dma_start(
                    lt_sb[:, lts:lts + 1, :],
                    lt_d.ap()[lts * 128:(lts + 1) * 128, :]
                    .rearrange("(t p) o -> p t o", p=128))

            rt_sb = wpool.tile([128, NOT, J], f32, name="rt_sb")
            out_sb = wpool.tile([128, NOT * BPC], f32, name="out_sb")

            def dma_x(xts_, blk_, sp):
                for bb in range(BBLK):
                    b = blk_ * BBLK + bb
                    nc.sync.dma_start(
                        xts_[bb][:, sp:sp + 1, :],
                        x_d.ap()[b, sp * 128:(sp + 1) * 128, :]
                        .rearrange("(t p) j -> p t j", p=128))

            def alloc_x(blk_):
                return [xpool.tile([128, NIT, J], f16,
                                   name=f"x_{blk_ * BBLK + bb}", tag="xt")
                        for bb in range(BBLK)]

            def load_rt_chunk(ot):
                nc.sync.dma_start(
                    rt_sb[:, ot:ot + 1, :],
                    rt_d.ap()[ot * 128:(ot + 1) * 128, :]
                    .rearrange("(t p) j -> p t j", p=128))

            # block 0 cold start: it-major JIT order so the first matmul can
            # start after ~3 DMAs. rt[0] slips in early (the first drain
            # needs it at ~10us and otherwise backs up PSUM -> stalls PE);
            # rt[1..7] trail the x/lt stream and land just-in-time for their
            # drains.
            xts = alloc_x(0)
            for sp in range(NIT):
                load_lt_chunk(sp)
                dma_x(xts, 0, sp)
                if sp == 0:
                    load_rt_chunk(0)
            for ot in range(1, NOT):
                load_rt_chunk(ot)
            for blk in range(NBLK):
                xts_next = alloc_x(blk + 1) if blk + 1 < NBLK else None
                for ot in range(NOT):
                    # pace the next block's DMA: two 256KB tiles per ot step
                    # instead of a 4MB burst (SBUF write contention with the
                    # PE's rhs stream)
                    if xts_next is not None:
                        dma_x(xts_next, blk + 1, ot)
                    pss = [
                        pspool.tile([128, J], f32,
                                    name=f"ps_{blk}_{ot}_{s}", tag="ps")
                        for s in range(BBLK)
                    ]
                    for it in range(NIT):
                        lhsT = lt_sb[:, it, ot * 128:(ot + 1) * 128]
                        for bb in range(BBLK):
                            for jc in range(NJC):
                                nc.tensor.matmul(
                                    pss[bb][:, jc * 512:(jc + 1) * 512],
                                    lhsT,
                                    xts[bb][:, it, jc * 512:(jc + 1) * 512],
                                    start=(it == 0),
                                    stop=(it == NIT - 1),
                                )
                    for bb in range(BBLK):
                        b = blk * BBLK + bb
                        # sc0 is pure scratch (only accum_out is consumed);
                        # bf16 halves the drain's SBUF write traffic
                        sc0 = spool.tile([128, J], mybir.dt.bfloat16,
                                         name=f"sc0_{b}_{ot}", tag="sc")
                        col = ot * BPC + b
                        # out = (ps * 1.0) * rt ; accum_out = sum_j(out)
                        nc.vector.scalar_tensor_tensor(
                            out=sc0[:],
                            in0=pss[bb][:],
                            scalar=1.0,
                            in1=rt_sb[:, ot, :],
                            op0=mybir.AluOpType.mult,
                            op1=mybir.AluOpType.mult,
                            accum_out=out_sb[:, col:col + 1],
                        )
                xts = xts_next if xts_next is not None else xts
            nc.sync.dma_start(out_d.ap(), out_sb[:])

    with tile.TileContext(nc) as tc:
        with (
            tc.tile_pool(name="w", bufs=1) as wpool,
            tc.tile_pool(name="xp", bufs=2 * BBLK) as xpool,
            tc.tile_pool(name="sc", bufs=4) as spool,
            tc.tile_pool(name="ps", bufs=4, space="PSUM") as pspool,
        ):
            loop = (tc.For_i(0, reps, 1) if reps is not None
                    else contextlib.nullcontext())
            with loop:
                body(tc, wpool, xpool, spool, pspool)
    nc.compile()
    _thin_sem_incs(nc)
    return nc


_NC_CACHE = []


def _get_nc():
    if not _NC_CACHE:
        _NC_CACHE.append(build_nc())
    return _NC_CACHE[0]


def make_in_maps(x: np.ndarray, L: np.ndarray, R: np.ndarray):
    xx = np.ascontiguousarray(x).astype(np.float16)
    lt = np.ascontiguousarray(L.T).astype(np.float16)
    rt = np.ascontiguousarray(R.T).astype(np.float32)
    return [
        {"x": xx[c * BPC:(c + 1) * BPC], "lt": lt, "rt": rt}
        for c in range(NCORES)
    ]


def assemble(results) -> np.ndarray:
    out = np.empty((B, O), np.float32)
    for c in range(NCORES):
        oc = results[c]["out"]                      # [128, NOT*BPC]
        t = oc.reshape(128, NOT, BPC)               # [p, ot, b]
        out[c * BPC:(c + 1) * BPC] = t.transpose(2, 1, 0).reshape(BPC, O)
    return out


def kernel(x: np.ndarray, L: np.ndarray, R: np.ndarray) -> np.ndarray:
    nc = _get_nc()
    res = run_bass_kernel_spmd(nc, make_in_maps(x, L, R),
                               core_ids=list(range(NCORES)))
    return assemble(res.results)
